# revision 1
# baseline (speedup 1.0000x reference)
"""Neighborhood (NATTEN-style) sparse attention, 5x5 window, on 8 trn2 NeuronCores.

Sharding: spatial (H) sequence-parallel across 8 cores. Each core gets a
16-row H slice of q plus a 20-row halo-extended slice of kv (K//2 = 2 halo
rows each side, clamped at the global border), computes projections,
windowed attention, output projection and residual locally, and the host
concatenates the 8 H-slices back into the full output.
"""
import numpy as np
from functools import partial

import jax
import jax.numpy as jnp

B, C, H, W, K = 2, 64, 128, 128, 5
S = 8              # cores
CH = H // S        # 16 rows per shard
PAD = K // 2       # 2
EXT = CH + 2 * PAD # 20 rows incl. halo
SCALE = C ** -0.5


def _window_idx(n, k):
    # NATTEN semantics: full kxk window, shifted (clamped) at borders.
    pad = k // 2
    start = np.clip(np.arange(n) - pad, 0, n - k)
    idx = start[:, None] + np.arange(k)
    rel = idx - np.arange(n)[:, None] + (k - 1)
    return idx.astype(np.int32), rel.astype(np.int32)


_IDX_H, _REL_H = _window_idx(H, K)   # (128, 5)
_IDX_W, _REL_W = _window_idx(W, K)   # (128, 5)
_EXT_STARTS = np.clip(np.arange(S) * CH - PAD, 0, H - EXT)  # (8,)

_IDX_LOC = np.stack([_IDX_H[s * CH:(s + 1) * CH] - _EXT_STARTS[s] for s in range(S)])  # (8, 16, 5)
_REL_LOC = np.stack([_REL_H[s * CH:(s + 1) * CH] for s in range(S)])                    # (8, 16, 5)


def _shard_body(qs, kvc, kve, iloc, rloc, Wq, bq, Wkv, bkv, rpb, Wp, bp, gamma):
    # qs: (B, CH, W, C) channels-last q slice; kvc: (B, CH, W, C) core kv slice
    # kve: (B, EXT, W, C) halo-extended kv slice
    qq = (qs @ Wq + bq) * SCALE
    kvp = kve @ Wkv + bkv
    kk, vv = kvp[..., :C], kvp[..., C:]

    def gather(x):  # (B, EXT, W, C) -> (B, CH, W, K, K, C)
        xw = x[:, iloc]                 # (B, CH, K, W, C)
        xw = xw[:, :, :, _IDX_W]        # (B, CH, K, W, K, C)
        return jnp.transpose(xw, (0, 1, 3, 2, 4, 5))

    kwin = gather(kk)
    vwin = gather(vv)

    attn = jnp.einsum('bijc,bijklc->bijkl', qq, kwin)              # (B, CH, W, K, K)
    bias = rpb[rloc[:, None, :, None], _REL_W[None, :, None, :]]   # (CH, W, K, K)
    attn = attn + bias
    attn = jax.nn.softmax(attn.reshape(B, CH, W, K * K), axis=-1).reshape(B, CH, W, K, K)

    out = jnp.einsum('bijkl,bijklc->bijc', attn, vwin)             # (B, CH, W, C)
    out = out @ Wp + bp
    out = gamma * out + kvc
    return out


_pmapped = jax.pmap(
    _shard_body,
    in_axes=(0, 0, 0, 0, 0, None, None, None, None, None, None, None, None),
)


def kernel(q, kv, Wq, bq, Wkv, bkv, rpb, Wp, bp, gamma):
    qx = np.ascontiguousarray(np.transpose(np.asarray(q), (0, 2, 3, 1)))    # (B,H,W,C)
    kvx = np.ascontiguousarray(np.transpose(np.asarray(kv), (0, 2, 3, 1)))  # (B,H,W,C)

    q_sh = np.stack([qx[:, s * CH:(s + 1) * CH] for s in range(S)])                    # (8,B,CH,W,C)
    kv_core = np.stack([kvx[:, s * CH:(s + 1) * CH] for s in range(S)])                # (8,B,CH,W,C)
    kv_ext = np.stack([kvx[:, _EXT_STARTS[s]:_EXT_STARTS[s] + EXT] for s in range(S)]) # (8,B,EXT,W,C)

    res = _pmapped(q_sh, kv_core, kv_ext, _IDX_LOC, _REL_LOC,
                   np.asarray(Wq), np.asarray(bq), np.asarray(Wkv), np.asarray(bkv),
                   np.asarray(rpb), np.asarray(Wp), np.asarray(bp), np.asarray(gamma))
    res = np.asarray(res)                                   # (8, B, CH, W, C)
    full = np.concatenate([res[s] for s in range(S)], axis=1)  # (B, H, W, C)
    return np.ascontiguousarray(np.transpose(full, (0, 3, 1, 2))).astype(np.float32)
